# revision 39
# baseline (speedup 1.0000x reference)
"""DiT attention kernel for 8 Trainium2 NeuronCores.

Sharding: tensor-parallel over head groups (4 groups of 4 heads) x
data-parallel over batch (2), giving 8 cores. Each core computes, for its
batch b and head group g:
    QT/KT = (x[b] @ W{q,k}[g].T + b)  in [head_dim, seq] layout
    partial rotary on global head 0 (cores with g==0; others get cos=1/sin=0)
    V in [seq, head_dim] layout (computed transposed then PE-transposed)
    S^T = K Q^T per head, P^T = exp(S^T/8), O^T = V_aug^T P^T (ones column
    in V gives softmax denominators for free), normalize, then the
    row-parallel Wo partial product out^T = Wo[g].T @ O^T.
The host sums the 4 partial out^T per batch, transposes, and adds bo.

Matmuls run in float32r (single-pass PE mode, ~1.5e-4 rel err).
"""

import sys

if "/opt/trn_rl_repo" not in sys.path:
    sys.path.insert(0, "/opt/trn_rl_repo")

from contextlib import ExitStack

import numpy as np

import concourse.bass as bass  # noqa: F401  (bass must import before bacc)
import concourse.mybir as mybir
import concourse.tile as tile
from concourse import bacc
from concourse.bass_utils import run_bass_kernel_spmd
from concourse.masks import make_identity

F32 = mybir.dt.float32
F32R = mybir.dt.float32r
BF16 = mybir.dt.bfloat16

B, S, DIM, HEADS, HEAD_DIM = 2, 2048, 1024, 16, 64
N_CORES = 8
TP = 4                      # head groups
GH = HEADS // TP            # heads per core (4)
GC = GH * HEAD_DIM          # cols per core slice (256)
EXP_FN = mybir.ActivationFunctionType.Exp


def _emit_body(nc, tc, ctx, d, phases=(1, 2, 3), inplace_rope=True):
    """Emit one pass of the kernel body. d = dict of dram APs."""
    consts = ctx.enter_context(tc.tile_pool(name="consts", bufs=1))

    ones4 = consts.tile([128, 4], F32, name="ones4", tag="ones4")
    nc.vector.memset(ones4[:], 1.0)

    # Persistent activations
    qt = [consts.tile([128, S], F32R, name=f"qt{i}", tag=f"qt{i}") for i in range(2)]
    kt = [consts.tile([128, S], F32R, name=f"kt{i}", tag=f"kt{i}") for i in range(2)]
    if not inplace_rope:
        qtr = consts.tile([64, S], F32R, name="qtr", tag="qtr")
        ktr = consts.tile([64, S], F32R, name="ktr", tag="ktr")
    # V in natural layout + ones column per head: head h at cols [65h, 65h+64],
    # col 65h+64 is 1.0 (gives softmax sums as output row 64 of the PV matmul).
    vstore = [consts.tile([128, 65 * GH], F32R, name=f"vs{i}", tag=f"vs{i}") for i in range(16)]
    otst = [consts.tile([128, S], F32R, name=f"ot{i}", tag=f"ot{i}") for i in range(2)]

    bq_sb = consts.tile([128, 2], F32, name="bq", tag="bq")
    bk_sb = consts.tile([128, 2], F32, name="bk", tag="bk")
    bvrep = consts.tile([128, GC], F32, name="bvrep", tag="bvrep")
    nc.sync.dma_start(out=bq_sb[:], in_=d["bq2"][:, :])
    nc.sync.dma_start(out=bk_sb[:], in_=d["bk2"][:, :])
    nc.sync.dma_start(out=bvrep[:], in_=d["bvrow"][:].to_broadcast([128, GC]))

    # ---------------- Phase 1: QKV projections (+rope) ---------------------
    with tc.tile_pool(name="xw", bufs=1) as xw:
        # interleave the wq-chunk and xt-chunk loads so the first Q matmuls
        # can start as soon as the first ~1MB lands
        with tc.tile_pool(name="wstream", bufs=9) as wsp:
            wq = [wsp.tile([128, GC], BF16, name="w", tag="w") for _ in range(8)]
            xt = [xw.tile([128, S], BF16, name=f"xt{k}", tag=f"xt{k}") for k in range(8)]
            for k in range(8):
                nc.sync.dma_start(out=wq[k][:], in_=d["wq"][128 * k : 128 * (k + 1), :])
                nc.sync.dma_start(out=xt[k][:], in_=d["xT"][128 * k : 128 * (k + 1), :])
            cos_sb = xw.tile([64, S], F32R, name="cos", tag="cos")
            sin_sb = xw.tile([64, S], F32R, name="sin", tag="sin")
            nc.sync.dma_start(out=cos_sb[:], in_=d["cosT"][:, :])
            nc.sync.dma_start(out=sin_sb[:], in_=d["sinT"][:, :])

            with tc.tile_pool(name="pq", bufs=2, space="PSUM") as pq:
                for nm, wd, w, bias_sb, dest in (
                    ("q", d["wq"], wq, bq_sb, qt),
                    ("k", d["wk"], None, bk_sb, kt),
                ):
                    if w is None:
                        w = [wsp.tile([128, GC], BF16, name="w", tag="w") for _ in range(8)]
                        for k in range(8):
                            nc.sync.dma_start(
                                out=w[k][:], in_=wd[128 * k : 128 * (k + 1), :]
                            )
                    for m in range(2):
                        ps = pq.tile([128, S], F32, name="pqkv", tag="pqkv")
                        for n in range(4):
                            for k in range(8):
                                nc.tensor.matmul(
                                    ps[:, 512 * n : 512 * (n + 1)],
                                    lhsT=w[k][:, 128 * m : 128 * (m + 1)],
                                    rhs=xt[k][:, 512 * n : 512 * (n + 1)],
                                    start=(k == 0),
                                    stop=(k == 7),
                                )
                        nc.vector.tensor_scalar_add(
                            out=dest[m][:], in0=ps[:], scalar1=bias_sb[:, m : m + 1]
                        )
                # V directly in natural [seq, vdim] layout (no PE transpose):
                # lhsT = x^T chunks, rhs = wv; bias broadcast along partitions
                wv = [wsp.tile([128, GC], BF16, name="w", tag="w") for _ in range(8)]
                for k in range(8):
                    nc.sync.dma_start(
                        out=wv[k][:], in_=d["wv"][128 * k : 128 * (k + 1), :]
                    )
                for blk in range(16):
                    nc.vector.tensor_copy(vstore[blk][:, 64 : 65 * GH : 65], ones4[:])
                for sc in range(16):
                    psv = pq.tile([128, GC], F32, name="pqkv", tag="pqkv")
                    for k in range(8):
                        nc.tensor.matmul(
                            psv[:],
                            lhsT=xt[k][:, 128 * sc : 128 * (sc + 1)],
                            rhs=wv[k][:, :],
                            start=(k == 0),
                            stop=(k == 7),
                        )
                    dst = (
                        vstore[sc][:, 0 : 65 * GH]
                        .rearrange("p (h c) -> p h c", h=GH)[:, :, 0:64]
                    )
                    nc.vector.tensor_add(
                        dst,
                        psv[:].rearrange("p (h c) -> p h c", h=GH),
                        bvrep[:].rearrange("p (h c) -> p h c", h=GH),
                    )

            # rotary on local head 0 (dims 0:64 of qt[0]/kt[0]); other
            # cores receive cos=1/sin=0 so this is an identity there.
            with tc.tile_pool(name="rope", bufs=1) as rp:
                for src, j in ((qt[0], 0), (kt[0], 1)):
                    dst = src[0:64, :] if inplace_rope else (qtr, ktr)[j][:]
                    sw = rp.tile([64, S], F32R, name=f"sw{j}", tag="ropetmp", bufs=2)
                    nc.sync.dma_start(out=sw[0:64:2, :], in_=src[1:64:2, :])
                    nc.sync.dma_start(out=sw[1:64:2, :], in_=src[0:64:2, :])
                    t1 = rp.tile([64, S], F32R, name=f"t1{j}", tag="ropetmp", bufs=2)
                    nc.vector.tensor_mul(t1[:], sw[:], sin_sb[:])
                    nc.vector.tensor_mul(dst, src[0:64, :], cos_sb[:])
                    nc.vector.tensor_add(dst, dst, t1[:])

    # ---------------- Phase 2: attention -----------------------------------
    if 2 in phases:
        with tc.tile_pool(name="ptp", bufs=6) as ptp, tc.tile_pool(
            name="stp", bufs=3, space="PSUM"
        ) as stp, tc.tile_pool(name="otp", bufs=2, space="PSUM") as otp, tc.tile_pool(
            name="nrm", bufs=3
        ) as nrm, tc.tile_pool(name="dscr", bufs=8, space="DRAM") as dscr:
            for p in range(2):  # head pair
                for qq in range(4):  # query quarter (512 wide)
                    ots = [otp.tile([128, 512], F32, name="otps", tag="otps") for _ in range(2)]
                    # sweep A: scores + exp for all 16 key blocks (PT fully
                    # materialized in SBUF); sweep B: the 32 PV matmuls.
                    # Keeps PE in one tile mode per sweep and lets the next
                    # quarter's exps overlap this quarter's PV matmuls.
                    pts = []
                    for blk in range(16):
                        st = stp.tile([128, 1024], F32, name="st", tag="st")
                        for hh in range(2):
                            rope = p == 0 and hh == 0 and not inplace_rope
                            k_ap = (
                                ktr[:, 128 * blk : 128 * (blk + 1)]
                                if rope
                                else kt[p][
                                    64 * hh : 64 * (hh + 1), 128 * blk : 128 * (blk + 1)
                                ]
                            )
                            q_ap = (
                                qtr[:, 512 * qq : 512 * (qq + 1)]
                                if rope
                                else qt[p][
                                    64 * hh : 64 * (hh + 1), 512 * qq : 512 * (qq + 1)
                                ]
                            )
                            nc.tensor.matmul(
                                st[:, 512 * hh : 512 * (hh + 1)],
                                lhsT=k_ap,
                                rhs=q_ap,
                                start=True,
                                stop=True,
                            )
                        pt = ptp.tile([128, 1024], F32R, name="pt", tag="pt", bufs=18)
                        nc.scalar.activation(pt[:], st[:], EXP_FN, scale=0.125)
                        pts.append(pt)
                    for blk in range(16):
                        for hh in range(2):
                            h = 2 * p + hh
                            nc.tensor.matmul(
                                ots[hh][0:65, :],
                                lhsT=vstore[blk][:, 65 * h : 65 * h + 65],
                                rhs=pts[blk][:, 512 * hh : 512 * (hh + 1)],
                                start=(blk == 0),
                                stop=(blk == 15),
                            )
                    # evict the un-normalized O^T + denominators to SBUF right
                    # away so the PSUM banks recycle without waiting on the
                    # normalization chain (which has two DRAM round-trips).
                    for hh in range(2):
                        ot_un = nrm.tile([128, 512], F32, name="ot_un", tag="ot_un", bufs=4)
                        nc.vector.tensor_copy(ot_un[0:65, :], ots[hh][0:65, :])
                        scr1 = dscr.tile([1, 512], F32, name="scr1", tag="scr1")
                        nc.sync.dma_start(out=scr1[:], in_=ot_un[64:65, :])
                        rst = nrm.tile([128, 4], F32, name="rst", tag="rst")
                        nc.sync.dma_start(
                            out=rst[:],
                            in_=scr1[:].rearrange("o (p f) -> (o p) f", p=128),
                        )
                        nc.vector.reciprocal(rst[:], rst[:])
                        scr2 = dscr.tile([1, 512], F32, name="scr2", tag="scr2")
                        nc.sync.dma_start(
                            out=scr2[:].rearrange("o (p f) -> (o p) f", p=128),
                            in_=rst[:],
                        )
                        bc = nrm.tile([64, 512], F32, name="bc", tag="bc")
                        nc.sync.dma_start(out=bc[:], in_=scr2[:].to_broadcast([64, 512]))
                        if hh == 0:
                            nc.vector.tensor_mul(
                                otst[p][0:64, 512 * qq : 512 * (qq + 1)],
                                ot_un[0:64, :],
                                bc[:].bitcast(F32R),
                            )
                        else:
                            # DVE cannot write partition base 64 from base-0
                            # inputs; go through a temp tile + sbuf->sbuf DMA.
                            tmp = nrm.tile([64, 512], F32R, name="tmp", tag="tmp")
                            nc.vector.tensor_mul(tmp[:], ot_un[0:64, :], bc[:].bitcast(F32R))
                            nc.sync.dma_start(
                                out=otst[p][64:128, 512 * qq : 512 * (qq + 1)],
                                in_=tmp[:],
                            )

    # ---------------- Phase 3: output projection (row-parallel partial) ----
    if 3 in phases:
        with tc.tile_pool(name="wop", bufs=1) as wop, tc.tile_pool(
            name="pw", bufs=4, space="PSUM"
        ) as pw:
            wo_sb = [wop.tile([128, DIM], F32R, name=f"wo{k}", tag=f"wo{k}") for k in range(2)]
            for k in range(2):
                nc.sync.dma_start(
                    out=wo_sb[k][:], in_=d["wo"][128 * k : 128 * (k + 1), :]
                )
            for m in range(8):
                for n in range(4):
                    ps = pw.tile([128, 512], F32, name="pwo", tag="pwo")
                    for k in range(2):
                        nc.tensor.matmul(
                            ps[:],
                            lhsT=wo_sb[k][:, 128 * m : 128 * (m + 1)],
                            rhs=otst[k][:, 512 * n : 512 * (n + 1)],
                            start=(k == 0),
                            stop=(k == 1),
                        )
                    ob = wop.tile([128, 512], F32, name="ob", tag="ob", bufs=4)
                    nc.vector.tensor_copy(ob[:], ps[:])
                    nc.sync.dma_start(out=d["outT4"][m, n], in_=ob[:])


def build_nc(reps: int = 1, phases=(1, 2, 3)):
    nc = bacc.Bacc(
        "TRN2", target_bir_lowering=False, debug=False, num_devices=N_CORES
    )
    d = {}
    d["xT"] = nc.dram_tensor("xT", [DIM, S], BF16, kind="ExternalInput").ap()
    for nm in ("wq", "wk", "wv"):
        d[nm] = nc.dram_tensor(nm, [DIM, GC], BF16, kind="ExternalInput").ap()
    for nm in ("bq2", "bk2"):
        d[nm] = nc.dram_tensor(nm, [128, 2], F32, kind="ExternalInput").ap()
    d["bvrow"] = nc.dram_tensor("bvrow", [1, GC], F32, kind="ExternalInput").ap()
    d["wo"] = nc.dram_tensor("wo", [GC, DIM], F32R, kind="ExternalInput").ap()
    d["cosT"] = nc.dram_tensor("cosT", [64, S], F32R, kind="ExternalInput").ap()
    d["sinT"] = nc.dram_tensor("sinT", [64, S], F32R, kind="ExternalInput").ap()
    # each (m, n) output tile is a contiguous 256KB block
    d["outT4"] = nc.dram_tensor(
        "outT4", [8, 4, 128, 512], F32, kind="ExternalOutput"
    ).ap()

    inplace_rope = reps == 1
    with tile.TileContext(nc) as tc, ExitStack() as ctx:
        if reps == 1:
            _emit_body(nc, tc, ctx, d, phases, inplace_rope)
        else:
            def body(_iv):
                with ExitStack() as inner:
                    _emit_body(nc, tc, inner, d, phases, inplace_rope)

            with tc.For_i(0, reps, 1) as iv:
                body(iv)
    nc.compile()
    return nc


def shard_inputs(x, cos, sin, Wq, bq, Wk, bk, Wv, bv, Wo, bo):
    """Build the per-core input maps (host-side sharding)."""
    bf16 = mybir.dt.np(mybir.dt.bfloat16)
    x = np.asarray(x, np.float32)
    cos = np.asarray(cos, np.float32).reshape(S, 64)
    sin = np.asarray(sin, np.float32).reshape(S, 64)
    sgn = np.tile(np.array([-1.0, 1.0], np.float32), 32)
    cosT = np.ascontiguousarray(cos.T)
    sinT = np.ascontiguousarray((sin * sgn).T)
    ones_cos = np.ones((64, S), np.float32)
    zero_sin = np.zeros((64, S), np.float32)
    xTs = [np.ascontiguousarray(x[b].T.astype(bf16)) for b in range(B)]

    in_maps = []
    for c in range(N_CORES):
        b, g = divmod(c, TP)
        sl = slice(GC * g, GC * (g + 1))
        m = {
            "xT": xTs[b],
            "wq": np.ascontiguousarray(np.asarray(Wq)[sl, :].T.astype(bf16)),
            "wk": np.ascontiguousarray(np.asarray(Wk)[sl, :].T.astype(bf16)),
            "wv": np.ascontiguousarray(np.asarray(Wv)[sl, :].T.astype(bf16)),
            "bq2": np.ascontiguousarray(np.asarray(bq, np.float32)[sl].reshape(2, 128).T),
            "bk2": np.ascontiguousarray(np.asarray(bk, np.float32)[sl].reshape(2, 128).T),
            "bvrow": np.asarray(bv, np.float32)[sl].reshape(1, GC).copy(),
            "wo": np.ascontiguousarray(np.asarray(Wo)[:, sl].T),
            "cosT": cosT if g == 0 else ones_cos,
            "sinT": sinT if g == 0 else zero_sin,
        }
        in_maps.append(m)
    return in_maps


def unshard_output(results, bo):
    bo = np.asarray(bo, np.float32)
    out = np.empty((B, S, DIM), np.float32)
    for b in range(B):
        acc = np.zeros((8, 4, 128, 512), np.float32)
        for g in range(TP):
            acc += results[TP * b + g]["outT4"]
        outT = acc.transpose(0, 2, 1, 3).reshape(DIM, S)
        out[b] = outT.T + bo
    return out


_NC_CACHE = {}


def get_nc(reps: int = 1, phases=(1, 2, 3)):
    key = (reps, tuple(phases))
    if key not in _NC_CACHE:
        _NC_CACHE[key] = build_nc(reps, phases)
    return _NC_CACHE[key]


def kernel(x, cos, sin, Wq, bq, Wk, bk, Wv, bv, Wo, bo, mask=None, _reps=1):
    nc = get_nc(_reps)
    in_maps = shard_inputs(x, cos, sin, Wq, bq, Wk, bk, Wv, bv, Wo, bo)
    res = run_bass_kernel_spmd(nc, in_maps, list(range(N_CORES)))
    return unshard_output(res.results, bo)

